# revision 1
# baseline (speedup 1.0000x reference)
import numpy as np
import jax
import jax.numpy as jnp
from jax import lax

# nn_HDRNet: B=2, 3x768x768 in/out, bilateral-grid HDRNet.
# Strategy: 8-way data parallel. Device i handles image i//4, row-slab (i%4)*192.
# The tiny low-res coeff branch is recomputed (replicated) on each device.
# The bilateral slice is computed gather-free via an exact piecewise-linear
# (hinge) decomposition in the guide value, so the (B,12,8,H,W) upsampled grid
# is never materialized.

B, H, W = 2, 768, 768
LOWRES, D = 256, 8
NDEV = 8
ROWS = 192  # rows per device


def _conv(x, w, b=None, stride=1, pad=1):
    y = lax.conv_general_dilated(x, w, (stride, stride), [(pad, pad), (pad, pad)],
                                 dimension_numbers=('NCHW', 'OIHW', 'NCHW'))
    if b is not None:
        y = y + b[None, :, None, None]
    return y


def _bn(x, g, b, m, v, eps=1e-5):
    inv = g * lax.rsqrt(v + eps)
    return x * inv[None, :, None, None] + (b - m * inv)[None, :, None, None]


def _convblock(x, p, name, stride=1):
    y = _conv(x, p[name + '_w'], None, stride, 1)
    y = _bn(y, p[name + '_g'], p[name + '_b'], p[name + '_m'], p[name + '_v'])
    return jax.nn.relu(y)


def _axis_tables(n_in, n_out):
    # torch bilinear align_corners=False source indices/weights (host, shape-only)
    src = (np.arange(n_out, dtype=np.float64) + 0.5) * (n_in / n_out) - 0.5
    src = np.maximum(src, 0.0)
    i0 = np.minimum(np.floor(src).astype(np.int32), n_in - 1)
    i1 = np.minimum(i0 + 1, n_in - 1)
    t = (src - i0).astype(np.float32)
    return i0, i1, t


# y-axis tables for upsampling the 8-row grid to 768 rows
_GY0, _GY1, _TY = _axis_tables(8, H)
# x-axis: segment structure (j0 constant per segment)
_J0, _J1, _TX = _axis_tables(8, W)
_SEGS = []
s = 0
for c in range(1, W + 1):
    if c == W or _J0[c] != _J0[s]:
        _SEGS.append((s, c, int(_J0[s]), int(_J1[s])))
        s = c


def _grid_branch(x_img, p):
    """x_img: (3, 768, 768) -> grid (12, 8, 8, 8) [ch, d, gy, gx]"""
    lowres = x_img[None, :, 1::3, 1::3]  # exact bilinear 768->256 (scale 3)
    f = _convblock(lowres, p, 'll1', 2)
    f = _convblock(f, p, 'll2', 2)
    f = _convblock(f, p, 'll3', 2)
    f = _convblock(f, p, 'll4', 2)             # (1,64,16,16)
    local = _convblock(f, p, 'hl1', 2)
    local = _convblock(local, p, 'hl2', 1)     # (1,64,8,8)
    gap = f.mean(axis=(2, 3))                  # (1,64)
    g1 = jax.nn.relu(gap @ p['fc1_w'].T + p['fc1_b'])
    g2 = jax.nn.relu(g1 @ p['fc2_w'].T + p['fc2_b'])
    high = local + g2[:, :, None, None]
    raw = _convblock(high, p, 'gr1', 1)
    raw = _conv(raw, p['gr2_w'], p['gr2_b'], 1, 0)   # (1,96,8,8)
    return raw[0].reshape(12, D, 8, 8)


def _shard_fn(x_img, row0, p, gy0, gy1, ty, txf):
    """x_img (3,768,768); row0 scalar; returns (3,192,768) for rows [row0,row0+192)."""
    grid = _grid_branch(x_img, p)                      # (12, 8, 8, 8)

    xs = lax.dynamic_slice(x_img, (0, row0, 0), (3, ROWS, W))  # (3,192,768)

    # full-res guide (1x1 convs) on the slab only
    h1 = jnp.einsum('oc,cyx->oyx', p['gd1_w'][:, :, 0, 0], xs) + p['gd1_b'][:, None, None]
    h1 = jax.nn.relu(h1)
    g = jnp.einsum('oc,cyx->yx', p['gd2_w'][:, :, 0, 0], h1) + p['gd2_b'][0]
    gs = jax.nn.sigmoid(g) * (D - 1)                   # (192,768) in (0,7)

    # y-interp grid rows for this slab
    my_gy0 = lax.dynamic_slice(gy0, (row0,), (ROWS,))
    my_gy1 = lax.dynamic_slice(gy1, (row0,), (ROWS,))
    my_ty = lax.dynamic_slice(ty, (row0,), (ROWS,))
    T0 = grid[:, :, my_gy0, :]                         # (12,8,192,8)
    T1 = grid[:, :, my_gy1, :]
    T = T0 + (T1 - T0) * my_ty[None, None, :, None]    # (12,8d,192,8gx)

    # hinge coefficients along d: PL(gs) = a0 + s0*gs + sum_k (s_k - s_{k-1}) relu(gs - k)
    slopes = T[:, 1:] - T[:, :-1]                      # (12,7,192,8)
    a0 = T[:, 0]                                       # (12,192,8)
    Wb = jnp.concatenate([
        a0[:, None], slopes[:, :1],
        slopes[:, 1:] - slopes[:, :-1],
    ], axis=1)                                         # (12,8m,192,8gx)

    # hinge basis of gs
    ks = jnp.arange(1, 7, dtype=jnp.float32)
    phi = jnp.concatenate([
        jnp.ones_like(gs)[None], gs[None],
        jax.nn.relu(gs[None] - ks[:, None, None]),
    ], axis=0)                                         # (8m,192,768)

    # x-interp via static segments (j0/j1 fixed per segment)
    outs = []
    for (c0, c1, g0, g1_) in _SEGS:
        ph = phi[:, :, c0:c1]                          # (8,192,seg)
        C0 = jnp.einsum('cmy,myx->cyx', Wb[:, :, :, g0], ph)
        C1 = jnp.einsum('cmy,myx->cyx', Wb[:, :, :, g1_], ph)
        t = txf[c0:c1][None, None, :]
        outs.append(C0 + (C1 - C0) * t)
    coeffs = jnp.concatenate(outs, axis=2)             # (12,192,768)

    mat = coeffs[:9].reshape(3, 3, ROWS, W)
    out = jnp.einsum('ciyx,iyx->cyx', mat, xs) + coeffs[9:12]
    return jnp.clip(out, 0.0, 1.0)


_PMAP = None


def _get_pmap():
    global _PMAP
    if _PMAP is None:
        _PMAP = jax.pmap(_shard_fn, in_axes=(0, 0, None, None, None, None, None))
    return _PMAP


def kernel(x, params):
    x = np.asarray(x, dtype=np.float32)
    p = {k: jnp.asarray(v) for k, v in params.items()}

    xs_dev = np.stack([x[i // 4] for i in range(NDEV)])          # (8,3,768,768)
    row0s = np.array([(i % 4) * ROWS for i in range(NDEV)], dtype=np.int32)

    fn = _get_pmap()
    out_sh = fn(jnp.asarray(xs_dev), jnp.asarray(row0s), p,
                jnp.asarray(_GY0), jnp.asarray(_GY1), jnp.asarray(_TY),
                jnp.asarray(_TX))
    out_sh = np.asarray(out_sh)                                   # (8,3,192,768)
    out = np.empty((B, 3, H, W), dtype=np.float32)
    for i in range(NDEV):
        out[i // 4, :, (i % 4) * ROWS:(i % 4 + 1) * ROWS, :] = out_sh[i]
    return out
